# revision 1
# baseline (speedup 1.0000x reference)
"""CrossAttention (softmax over query axis + row renorm) on 8 trn2 cores.

Sharding: core c -> batch b = c//2, head-group g = c%2 (4 of 8 heads).
Each core: Q/K/V projections for its heads (full q/k), attention with the
q-axis softmax done locally in S^T = [k, q] layout, W0 partial product,
2-core ReduceScatter over the pair (summing head groups, splitting q),
then residual + W1 on its query half. Output rows [g*1024:(g+1)*1024] of
batch b.

Shapes (hardcoded): B=4, NQ=NK=2048, D=512, H=8, DH=64.
"""

import sys

for p in ("/opt/trn_rl_repo", "/opt/pypackages"):
    if p not in sys.path:
        sys.path.insert(0, p)

import numpy as np
from contextlib import ExitStack

import concourse.bass as bass
import concourse.mybir as mybir
import concourse.tile as tile
from concourse.bass_utils import run_bass_kernel_spmd

B, NQ, NK, D, H, DH = 4, 2048, 2048, 512, 8, 64
HG = 4          # heads per core (head-group size)
GCOL = HG * DH  # 256 projection columns per core
QH = NQ // 2    # query rows per core after reduce-scatter
P = 128
F32 = mybir.dt.float32
F32R = mybir.dt.float32r

USE_F32R = True  # float32r streams 1 col/cycle vs fp32's 4 (tf32-like rounding)
LINEARIZE = True  # serialize scheduling: walrus encodes only 1 sync wait per
                  # engine instruction on this toolchain; the overlap-scheduled
                  # build trips 'Too many sync wait commands' in codegen
MDT = F32R if USE_F32R else F32


def _mm(ap):
    return ap


def build_kernel():
    nc = bass.Bass(num_devices=8)

    xt_d = nc.dram_tensor("xt", [D, NQ], MDT, kind="ExternalInput")
    et_d = nc.dram_tensor("et", [D, NK], MDT, kind="ExternalInput")
    wq_d = nc.dram_tensor("wq", [D, GCOL], MDT, kind="ExternalInput")
    wk_d = nc.dram_tensor("wk", [D, GCOL], MDT, kind="ExternalInput")
    wv_d = nc.dram_tensor("wv", [D, GCOL], MDT, kind="ExternalInput")
    w0_d = nc.dram_tensor("w0", [GCOL, D], MDT, kind="ExternalInput")
    w1_d = nc.dram_tensor("w1", [D, D], MDT, kind="ExternalInput")
    b0_d = nc.dram_tensor("b0", [D], F32, kind="ExternalInput")
    b1_d = nc.dram_tensor("b1", [D], F32, kind="ExternalInput")
    xh_d = nc.dram_tensor("xh", [D, QH], F32, kind="ExternalInput")
    out_d = nc.dram_tensor("out", [QH, D], F32, kind="ExternalOutput")

    KC = D // P      # 4 contraction subtiles of 128
    NKB = NK // P    # 16 key blocks
    NCH = NK // 512  # 4 free-dim chunks of 512 over q/k

    with tile.TileContext(nc, linearize=LINEARIZE) as tc, ExitStack() as ctx, \
            nc.allow_low_precision(reason="float32r is 4-byte; matmul rounding"):
        # bufs=1 pool; tags shared between phase-disjoint tiles to fit SBUF
        mem = ctx.enter_context(tc.tile_pool(name="mem", bufs=1))
        work = ctx.enter_context(tc.tile_pool(name="work", bufs=2))
        single = ctx.enter_context(tc.tile_pool(name="single", bufs=1))
        small = ctx.enter_context(tc.tile_pool(name="small", bufs=4))
        # spsum 2x[128,1024] = 4 banks, opsum [65,2048] = 4 banks -> 8 total.
        # Projection-phase psums borrow the spsum tag (phase-disjoint).
        ps2 = ctx.enter_context(tc.tile_pool(name="ps2", bufs=2, space="PSUM"))
        psb = ctx.enter_context(tc.tile_pool(name="psb", bufs=1, space="PSUM"))
        dram = ctx.enter_context(tc.tile_pool(name="dram", bufs=1, space="DRAM"))

        # ---- load inputs -------------------------------------------------
        xt = mem.tile([P, KC, NQ], MDT, tag="bigA")
        nc.sync.dma_start(xt, xt_d.rearrange("(c p) q -> p c q", p=P))
        et = mem.tile([P, KC, NK], MDT, tag="bigB")
        nc.sync.dma_start(et, et_d.rearrange("(c p) q -> p c q", p=P))
        wq = mem.tile([P, KC, GCOL], MDT, tag="t16c")
        nc.sync.dma_start(wq, wq_d.rearrange("(c p) m -> p c m", p=P))
        wk = mem.tile([P, KC, GCOL], MDT, tag="wk")
        nc.sync.dma_start(wk, wk_d.rearrange("(c p) m -> p c m", p=P))
        wv = mem.tile([P, KC, GCOL], MDT, tag="wv")
        nc.sync.dma_start(wv, wv_d.rearrange("(c p) m -> p c m", p=P))
        w0 = mem.tile([DH, HG, D], MDT, tag="w0")
        nc.sync.dma_start(w0, w0_d.rearrange("(h p) d -> p h d", p=DH))
        w1 = mem.tile([P, KC, D], MDT, tag="w1")
        nc.sync.dma_start(w1, w1_d.rearrange("(c p) d -> p c d", p=P))
        # DVE in-place x1.0 "rounding" passes: make DVE the single producer
        # proc of every matmul operand (fp32r fused-LDW matmuls carry only
        # one sync wait, so each matmul may depend on at most one engine).
        for t in (xt, et, wq, wk, wv, w0, w1):
            nc.vector.tensor_scalar_mul(t, t, 1.0)
        b0s = mem.tile([P, KC], F32, tag="b0")
        nc.sync.dma_start(b0s, b0_d.rearrange("(c p) -> p c", p=P))
        b1b = mem.tile([P, D], F32, tag="b1")
        nc.gpsimd.dma_start(b1b, b1_d[:].partition_broadcast(P))

        # ---- projections: QT/KT [128(head pair), 2, N*], V [128, 16, GCOL]
        qt = mem.tile([P, 2, NQ], MDT, tag="qt")
        kt = mem.tile([P, 2, NK], MDT, tag="kt")
        for mc in range(2):        # two head-pairs: 128 cols of wq each
            for nch in range(NCH):
                pq = ps2.tile([P, 512], F32, tag="spsum", name="pq")
                pk = ps2.tile([P, 512], F32, tag="spsum", name="pk")
                for kc in range(KC):
                    nc.tensor.matmul(
                        pq, _mm(wq[:, kc, mc * P:(mc + 1) * P]),
                        _mm(xt[:, kc, nch * 512:(nch + 1) * 512]),
                        start=(kc == 0), stop=(kc == KC - 1))
                for kc in range(KC):
                    nc.tensor.matmul(
                        pk, _mm(wk[:, kc, mc * P:(mc + 1) * P]),
                        _mm(et[:, kc, nch * 512:(nch + 1) * 512]),
                        start=(kc == 0), stop=(kc == KC - 1))
                nc.vector.tensor_copy(qt[:, mc, nch * 512:(nch + 1) * 512], pq)
                nc.vector.tensor_copy(kt[:, mc, nch * 512:(nch + 1) * 512], pk)

        v = mem.tile([P, NKB, GCOL], MDT, tag="v")
        for kb in range(NKB):
            pv = ps2.tile([P, GCOL], F32, tag="spsum", name="pv")
            for kc in range(KC):
                nc.tensor.matmul(
                    pv, _mm(et[:, kc, kb * P:(kb + 1) * P]),
                    _mm(wv[:, kc, :]),
                    start=(kc == 0), stop=(kc == KC - 1))
            nc.vector.tensor_copy(v[:, kb, :], pv)

        # Absorb outstanding DVE-side psum-slot releases into PE's vector
        # clock: fp32r fused-LDW matmuls can carry only ONE sync wait, so any
        # slot whose last accessor was DVE must be re-observed via these tiny
        # matmuls before the attention loop's matmuls touch those slots.
        scr_f = mem.tile([DH + 1, DH], F32, tag="scrf")
        nc.vector.memset(scr_f, 1.0)
        scr = mem.tile([1, 8], MDT, tag="scr")
        nc.vector.tensor_scalar_mul(scr, scr_f[0:1, 0:8], 1.0)
        ones_t = mem.tile([DH + 1, DH], MDT, tag="ones")
        nc.vector.tensor_scalar_mul(ones_t, scr_f, 1.0)
        for _i in range(2):
            dmy = ps2.tile([1, 8], F32, tag="spsum", name="dmy")
            nc.tensor.matmul(dmy, _mm(scr[0:1, 0:1]), _mm(scr), start=True, stop=True)
        dmy2 = psb.tile([1, 8], F32, tag="opsum", name="dmy2")
        nc.tensor.matmul(dmy2, _mm(scr[0:1, 0:1]), _mm(scr), start=True, stop=True)

        # ---- attention per head ------------------------------------------
        # S^T[k,q] = K_h @ Q_h^T; softmax over q = free axis per partition;
        # no max-subtraction (|s| <~ 10 so exp is fp32-safe). D1[k] = rowsum
        # comes free via accum_out. 1/D1 folds into V; a 65th lhsT column of
        # 1/D1 makes psum row 64 the per-q renorm denominator.
        ot = mem.tile([DH, HG, NQ], MDT, tag="bigA")  # reuses xt's slot
        for h in range(HG):
            hp, off = h // 2, (h % 2) * DH
            po = psb.tile([DH + 1, NK], F32, tag="opsum", name="po")
            for kb in range(NKB):
                e = work.tile([P, NK], MDT, tag="e")
                d1a = small.tile([P, 2], F32, tag="d1a")
                for ck in range(2):
                    ps = ps2.tile([P, NK // 2], F32, tag="spsum", name="ps")
                    for nch in range(2):
                        nc.tensor.matmul(
                            ps[:, nch * 512:(nch + 1) * 512],
                            _mm(kt[off:off + DH, hp, kb * P:(kb + 1) * P]),
                            _mm(qt[off:off + DH, hp,
                                   ck * 1024 + nch * 512:ck * 1024 + (nch + 1) * 512]),
                            start=True, stop=True)
                    nc.scalar.activation(e[:, ck * 1024:(ck + 1) * 1024], ps,
                                         mybir.ActivationFunctionType.Exp,
                                         accum_out=d1a[:, ck:ck + 1])
                rd = small.tile([P, 1], F32, tag="rd")
                nc.vector.tensor_tensor(rd, d1a[:, 0:1], d1a[:, 1:2],
                                        mybir.AluOpType.add)
                nc.vector.reciprocal(rd, rd)
                vaug = small.tile([P, DH + 1], MDT, tag="vaug")
                nc.scalar.activation(vaug[:, :DH], v[:, kb, h * DH:(h + 1) * DH],
                                     mybir.ActivationFunctionType.Copy, scale=rd)
                nc.scalar.copy(vaug[:, DH:DH + 1], rd)
                for nch in range(NCH):
                    nc.tensor.matmul(
                        po[:, nch * 512:(nch + 1) * 512],
                        _mm(vaug), _mm(e[:, nch * 512:(nch + 1) * 512]),
                        start=(kb == 0), stop=(kb == NKB - 1))
            # Drain po on ACT so the psum slot's release is visible through
            # the same ACT wait the next head's PV matmul already needs.
            poc = single.tile([DH + 1, NK], MDT, tag="poc")
            nc.scalar.copy(poc, po)
            # renormalize: O~ = O_raw / denom2. Reciprocal on the denom row,
            # broadcast across 64 partitions with a K=1 ones-matmul (operands
            # at partition 64), multiply into fp32, then round to f32r
            # (TensorTensor can't emit f32r, TensorScalar can).
            nc.vector.reciprocal(poc[DH:DH + 1, :], poc[DH:DH + 1, :])
            for ck in range(NCH):
                rb = ps2.tile([DH, 512], F32, tag="spsum", name="rb")
                nc.tensor.matmul(rb, _mm(ones_t[DH:DH + 1, :]),
                                 _mm(poc[DH:DH + 1, ck * 512:(ck + 1) * 512]),
                                 start=True, stop=True)
                otf = work.tile([DH, 512], F32, tag="fout", name="otf")
                nc.vector.tensor_tensor(otf, poc[:DH, ck * 512:(ck + 1) * 512],
                                        rb, mybir.AluOpType.mult)
                nc.vector.tensor_scalar_mul(ot[:, h, ck * 512:(ck + 1) * 512],
                                            otf, 1.0)

        # absorb attention-era slot releases before the W0 matmuls
        for _i in range(2):
            dmy3 = ps2.tile([1, 8], F32, tag="spsum", name="dmy3")
            nc.tensor.matmul(dmy3, _mm(scr[0:1, 0:1]), _mm(scr), start=True, stop=True)

        # ---- W0 partial: A^T[D, q] = sum_h W0_h^T @ O~_h^T (+ b0) --------
        at = mem.tile([P, KC, NQ], F32, tag="bigB")  # reuses et's slot
        a_part = dram.tile([2, D, QH], F32)
        for dc in range(KC):
            for nch in range(NCH):
                pa = ps2.tile([P, 512], F32, tag="spsum", name="pa")
                for h in range(HG):
                    nc.tensor.matmul(
                        pa, _mm(w0[:, h, dc * P:(dc + 1) * P]),
                        _mm(ot[:, h, nch * 512:(nch + 1) * 512]),
                        start=(h == 0), stop=(h == HG - 1))
                nc.vector.tensor_scalar(at[:, dc, nch * 512:(nch + 1) * 512], pa,
                                        scalar1=b0s[:, dc:dc + 1],
                                        scalar2=None, op0=mybir.AluOpType.add)
        for s in range(2):  # one DMA per RS slot keeps the collective's waits low
            nc.sync.dma_start(
                a_part[s].rearrange("(c p) q -> p c q", p=P),
                at[:, :, s * QH:(s + 1) * QH])

        a_rs = dram.tile([D, QH], F32)
        nc.gpsimd.collective_compute(
            "ReduceScatter", mybir.AluOpType.add,
            replica_groups=[[0, 1], [2, 3], [4, 5], [6, 7]],
            ins=[a_part.opt()], outs=[a_rs.opt()])

        # ---- residual + W1 on local q-half -------------------------------
        # RS rank r gets slot r = q rows [r*QH,(r+1)*QH); rank == head-group
        # g, so the host passes the matching X^T half per core ("xh").
        ars = mem.tile([P, KC, QH], F32, tag="ars")
        nc.sync.dma_start(ars, a_rs[:].rearrange("(c p) q -> p c q", p=P))
        xh = mem.tile([P, KC, QH], F32, tag="xh")
        nc.sync.dma_start(xh, xh_d.rearrange("(c p) q -> p c q", p=P))
        rt = mem.tile([P, KC, QH], MDT, tag="kt")  # reuses kt's slot
        for dc in range(KC):  # R^T = X^T - (A^T + b0), rounded for the matmul
            nc.vector.tensor_tensor(ars[:, dc, :], xh[:, dc, :], ars[:, dc, :],
                                    mybir.AluOpType.subtract)
            nc.vector.tensor_scalar_mul(rt[:, dc, :], ars[:, dc, :], 1.0)
        for mq in range(QH // P):
            pf = ps2.tile([P, D], F32, tag="spsum", name="pf")
            for kc in range(KC):
                nc.tensor.matmul(pf, _mm(rt[:, kc, mq * P:(mq + 1) * P]),
                                 _mm(w1[:, kc, :]),
                                 start=(kc == 0), stop=(kc == KC - 1))
            fo = work.tile([P, D], F32, tag="fout", name="fo")
            nc.vector.tensor_tensor(fo, pf, b1b, mybir.AluOpType.add)
            nc.sync.dma_start(out_d[mq * P:(mq + 1) * P, :], fo)

    _strip_redundant_self_waits(nc)
    _keep_latest_wait_only(nc)
    return nc


def _keep_latest_wait_only(nc):
    """Under linearize=True every instruction syncs on its predecessor, so
    waits on earlier instructions are transitively covered; keep only the
    wait whose target is latest in program order (walrus on this toolchain
    encodes a single sync wait per engine instruction)."""
    insts = []
    for blk in nc.m.functions[0].blocks:
        insts.extend(blk.instructions)
    pos = {}
    cums = {}
    for i, inst in enumerate(insts):
        si = getattr(inst, 'sync_info', None)
        if si and si.on_update:
            for u in si.on_update:
                cums[u.ant_name] = cums.get(u.ant_name, 0) + u.update_value
                pos[(u.ant_name, cums[u.ant_name])] = i
    for inst in insts:
        si = getattr(inst, 'sync_info', None)
        if si is None or not si.on_wait or len(si.on_wait) < 2:
            continue
        ws = list(si.on_wait)
        ws.sort(key=lambda w: pos.get((w.ant_name, w.wait_value), -1))
        si.on_wait = [ws[-1]]


_ENGINE_SEMS = {"PE_44", "Activation_44", "DVE_44", "Pool_44", "SP_44"}


def _strip_redundant_self_waits(nc):
    """Drop same-engine self waits: these engines retire instructions in
    pc order (strict FIFO queues; PE matmul completions are pc-monotone),
    so an instruction never needs a semaphore wait on its own engine's
    earlier non-DMA instruction. Needed because walrus encodes very few
    sync waits per instruction (1 for fused-LDW matmuls and ACTIVATE)."""
    insts = []
    for blk in nc.m.functions[0].blocks:
        insts.extend(blk.instructions)
    # per-sem cumulative tick -> instruction
    ticks = {s: {} for s in _ENGINE_SEMS}
    cums = {s: 0 for s in _ENGINE_SEMS}
    for inst in insts:
        si = getattr(inst, 'sync_info', None)
        if si and si.on_update:
            for u in si.on_update:
                if u.ant_name in _ENGINE_SEMS:
                    cums[u.ant_name] += u.update_value
                    ticks[u.ant_name][cums[u.ant_name]] = inst
    for inst in insts:
        tname = type(inst).__name__
        if 'DMA' in tname or 'Collective' in tname:
            continue
        si = getattr(inst, 'sync_info', None)
        if si is None or not si.on_wait or len(si.on_wait) < 2:
            continue
        my_engine = getattr(inst, 'engine', None)
        kept = []
        for w in si.on_wait:
            tgt = ticks.get(w.ant_name, {}).get(w.wait_value)
            same_engine = (
                tgt is not None
                and 'DMA' not in type(tgt).__name__
                and 'Collective' not in type(tgt).__name__
                and getattr(tgt, 'engine', None) == my_engine
            )
            if not same_engine:
                kept.append(w)
        if len(kept) != len(si.on_wait):
            si.on_wait = kept


def make_in_maps(init_query, embedding, Wq, Wk, Wv, W0, b0, W1, b1):
    init_query = np.asarray(init_query, np.float32)
    embedding = np.asarray(embedding, np.float32)
    Wq, Wk, Wv = (np.asarray(a, np.float32) for a in (Wq, Wk, Wv))
    W0, W1 = np.asarray(W0, np.float32), np.asarray(W1, np.float32)
    in_maps = []
    for c in range(8):
        b, g = c // 2, c % 2
        in_maps.append({
            "xt": np.ascontiguousarray(init_query[b].T),
            "et": np.ascontiguousarray(embedding[b].T),
            "wq": np.ascontiguousarray(Wq[:, g * GCOL:(g + 1) * GCOL]),
            "wk": np.ascontiguousarray(Wk[:, g * GCOL:(g + 1) * GCOL]),
            "wv": np.ascontiguousarray(Wv[:, g * GCOL:(g + 1) * GCOL]),
            "w0": np.ascontiguousarray(W0[g * GCOL:(g + 1) * GCOL, :]),
            "w1": W1,
            "b0": np.asarray(b0, np.float32),
            "b1": np.asarray(b1, np.float32),
            "xh": np.ascontiguousarray(init_query[b].T[:, g * QH:(g + 1) * QH]),
        })
    return in_maps


def kernel(init_query, embedding, Wq, Wk, Wv, W0, b0, W1, b1):
    nc = build_kernel()
    in_maps = make_in_maps(init_query, embedding, Wq, Wk, Wv, W0, b0, W1, b1)
    res = run_bass_kernel_spmd(nc, in_maps, list(range(8)))
    out = np.empty((B, NQ, D), np.float32)
    for c in range(8):
        b, g = c // 2, c % 2
        out[b, g * QH:(g + 1) * QH, :] = res.results[c]["out"]
    return out



# revision 2
# speedup vs baseline: 1.1927x; 1.1927x over previous
"""CrossAttention (softmax over query axis + row renorm) on 8 trn2 cores.

Wire-optimized vs v1: the axon tunnel (~30 MB/s) dominates wall time, so
inputs are shipped exactly once in bf16 (2.43 MB/core = unique/8) and
reconstructed on device with three AllGathers over NeuronLink:
  segA (pair AG   [2b,2b+1]): this core's q-half of x^T[b] and e^T[b]
  segB (AG-4 [g,2+g,4+g,6+g]): one of {Wq,Wk,Wv,W0} column-halves for g
  segC (AG-8): 1/8 of W1 + b0 + b1
Sharding: core c -> batch b = c//2, head-group g = c%2 (4 of 8 heads).
Compute: Q/K/V projections (bf16 matmuls), attention with q-axis softmax
in S^T=[k,q] layout, W0 partial folded with the residual (each core
contributes 0.5*x - 0.5*b0 - A_part so the 2-core bf16 ReduceScatter
yields R = x - (O@W0 + b0) directly), then W1 on the local q-half.
Output: bf16 [QH, D] = rows [g*1024:(g+1)*1024] of batch b.

Shapes (hardcoded): B=4, NQ=NK=2048, D=512, H=8, DH=64.
"""

import sys

for p in ("/opt/trn_rl_repo", "/opt/pypackages"):
    if p not in sys.path:
        sys.path.insert(0, p)

import numpy as np
import ml_dtypes
from contextlib import ExitStack

import concourse.bass as bass
import concourse.mybir as mybir
import concourse.tile as tile
from concourse.bass_utils import run_bass_kernel_spmd

B, NQ, NK, D, H, DH = 4, 2048, 2048, 512, 8, 64
HG = 4          # heads per core (head-group size)
GCOL = HG * DH  # 256 projection columns per core
QH = NQ // 2    # query rows per core after reduce-scatter
P = 128
F32 = mybir.dt.float32
F32R = mybir.dt.float32r
BF16 = mybir.dt.bfloat16
NPBF = ml_dtypes.bfloat16

SEGA = 2 * D * QH            # 1,048,576: x^T half + e^T half
SEGB = D * GCOL              # 131,072: one of wq/wk/wv/w0 for group g
SEGC = (D * D + 2 * D) // 8  # 32,896: 1/8 of (w1, b0, b1)
NIN = SEGA + SEGB + SEGC     # 1,212,544 bf16 elems per core

LINEARIZE = True  # serialize scheduling: walrus encodes only 1 sync wait per
                  # engine instruction on this toolchain; the overlap-scheduled
                  # build trips 'Too many sync wait commands' in codegen


def _mm(ap):
    return ap


def build_kernel():
    nc = bass.Bass(num_devices=8)

    in_d = nc.dram_tensor("inp", [NIN], BF16, kind="ExternalInput")
    out_d = nc.dram_tensor("out", [QH, D], BF16, kind="ExternalOutput")

    KC = D // P      # 4 contraction subtiles of 128
    NKB = NK // P    # 16 key blocks
    NCH = NK // 512  # 4 free-dim chunks of 512 over q/k

    with tile.TileContext(nc, linearize=LINEARIZE) as tc, ExitStack() as ctx, \
            nc.allow_low_precision(reason="bf16 wire/matmul; tolerance 2e-2"):
        mem = ctx.enter_context(tc.tile_pool(name="mem", bufs=1))
        work = ctx.enter_context(tc.tile_pool(name="work", bufs=2))
        single = ctx.enter_context(tc.tile_pool(name="single", bufs=1))
        small = ctx.enter_context(tc.tile_pool(name="small", bufs=4))
        ps2 = ctx.enter_context(tc.tile_pool(name="ps2", bufs=2, space="PSUM"))
        psb = ctx.enter_context(tc.tile_pool(name="psb", bufs=1, space="PSUM"))
        dram = ctx.enter_context(tc.tile_pool(name="dram", bufs=1, space="DRAM"))

        # ---- distribute the packed input with AllGathers -----------------
        stA = mem.tile([P, SEGA // P], BF16, tag="stA")
        stB = mem.tile([P, SEGB // P], BF16, tag="stB")
        stC = mem.tile([P, SEGC // P], BF16, tag="stC")
        nc.sync.dma_start(stA, in_d[0:SEGA].rearrange("(p f) -> p f", p=P))
        nc.sync.dma_start(stB, in_d[SEGA:SEGA + SEGB].rearrange("(p f) -> p f", p=P))
        nc.sync.dma_start(stC, in_d[SEGA + SEGB:NIN].rearrange("(p f) -> p f", p=P))
        aginA = dram.tile([SEGA], BF16, tag="aginA")
        aginB = dram.tile([SEGB], BF16, tag="aginB")
        aginC = dram.tile([SEGC], BF16, tag="aginC")
        nc.sync.dma_start(aginA.rearrange("(p f) -> p f", p=P), stA)
        nc.sync.dma_start(aginB.rearrange("(p f) -> p f", p=P), stB)
        nc.sync.dma_start(aginC.rearrange("(p f) -> p f", p=P), stC)
        agA = dram.tile([2 * SEGA], BF16, tag="agA")
        agB = dram.tile([4 * SEGB], BF16, tag="agB")
        agC = dram.tile([8 * SEGC], BF16, tag="agC")
        nc.gpsimd.collective_compute(
            "AllGather", mybir.AluOpType.bypass,
            replica_groups=[[0, 1], [2, 3], [4, 5], [6, 7]],
            ins=[aginA.opt()], outs=[agA.opt()])
        nc.gpsimd.collective_compute(
            "AllGather", mybir.AluOpType.bypass,
            replica_groups=[[0, 2, 4, 6], [1, 3, 5, 7]],
            ins=[aginB.opt()], outs=[agB.opt()])
        nc.gpsimd.collective_compute(
            "AllGather", mybir.AluOpType.bypass,
            replica_groups=[[0, 1, 2, 3, 4, 5, 6, 7]],
            ins=[aginC.opt()], outs=[agC.opt()])

        # ---- unpack gathered tensors into SBUF ---------------------------
        # agA slot s (pair rank s = head-group s) holds q columns
        # [s*1024,(s+1)*1024) of x^T[b] then e^T[b], each (c p q) c=4 p=128.
        XHALF = D * QH
        xt = mem.tile([P, KC, NQ], BF16, tag="bigA")
        et = mem.tile([P, KC, NK], BF16, tag="bigB")
        for s in range(2):
            o = s * SEGA
            nc.sync.dma_start(
                xt[:, :, s * QH:(s + 1) * QH],
                agA[o:o + XHALF].rearrange("(c p q) -> p c q", p=P, q=QH))
            nc.sync.dma_start(
                et[:, :, s * QH:(s + 1) * QH],
                agA[o + XHALF:o + 2 * XHALF].rearrange("(c p q) -> p c q", p=P, q=QH))
        # agB slots: 0=wq_h, 1=wk_h, 2=wv_h (c p m), 3=w0_h (h p d)
        wq = mem.tile([P, KC, GCOL], BF16, tag="wq")
        wk = mem.tile([P, KC, GCOL], BF16, tag="wk")
        wv = mem.tile([P, KC, GCOL], BF16, tag="wv")
        nc.sync.dma_start(wq, agB[0:SEGB].rearrange("(c p m) -> p c m", p=P, m=GCOL))
        nc.sync.dma_start(wk, agB[SEGB:2 * SEGB].rearrange("(c p m) -> p c m", p=P, m=GCOL))
        nc.sync.dma_start(wv, agB[2 * SEGB:3 * SEGB].rearrange("(c p m) -> p c m", p=P, m=GCOL))
        w0 = mem.tile([DH, HG, D], BF16, tag="w0")
        nc.sync.dma_start(w0, agB[3 * SEGB:4 * SEGB].rearrange("(h p d) -> p h d", p=DH, d=D))
        # agC stream: w1 (c p d), b0 (c p), b1 (d)
        w1 = mem.tile([P, KC, D], BF16, tag="w1")
        nc.sync.dma_start(w1, agC[0:D * D].rearrange("(c p d) -> p c d", p=P, d=D))
        b0s = mem.tile([P, KC], BF16, tag="b0")
        nc.sync.dma_start(b0s, agC[D * D:D * D + D].rearrange("(c p) -> p c", p=P))
        b1b = mem.tile([P, D], BF16, tag="b1")
        nc.gpsimd.dma_start(b1b, agC[D * D + D:D * D + 2 * D].partition_broadcast(P))

        # DVE in-place x1.0 "rounding" passes: make DVE the single producer
        # proc of every matmul operand (fused-LDW matmuls carry only one
        # sync wait, so each matmul may depend on at most one engine).
        for t in (xt, et, wq, wk, wv, w0, w1):
            nc.vector.tensor_scalar_mul(t, t, 1.0)
        b0h = mem.tile([P, KC], F32, tag="b0h")  # 0.5*b0 per partition
        nc.vector.tensor_scalar_mul(b0h, b0s, 0.5)

        # ---- projections: QT/KT [128(head pair), 2, N*], V [128, 16, GCOL]
        qt = mem.tile([P, 2, NQ], BF16, tag="qt")
        kt = mem.tile([P, 2, NK], BF16, tag="kt")
        for mc in range(2):        # two head-pairs: 128 cols of wq each
            for nch in range(NCH):
                pq = ps2.tile([P, 512], F32, tag="spsum", name="pq")
                pk = ps2.tile([P, 512], F32, tag="spsum", name="pk")
                for kc in range(KC):
                    nc.tensor.matmul(
                        pq, _mm(wq[:, kc, mc * P:(mc + 1) * P]),
                        _mm(xt[:, kc, nch * 512:(nch + 1) * 512]),
                        start=(kc == 0), stop=(kc == KC - 1))
                for kc in range(KC):
                    nc.tensor.matmul(
                        pk, _mm(wk[:, kc, mc * P:(mc + 1) * P]),
                        _mm(et[:, kc, nch * 512:(nch + 1) * 512]),
                        start=(kc == 0), stop=(kc == KC - 1))
                nc.vector.tensor_copy(qt[:, mc, nch * 512:(nch + 1) * 512], pq)
                nc.vector.tensor_copy(kt[:, mc, nch * 512:(nch + 1) * 512], pk)

        v = mem.tile([P, NKB, GCOL], BF16, tag="v")
        for kb in range(NKB):
            pv = ps2.tile([P, GCOL], F32, tag="spsum", name="pv")
            for kc in range(KC):
                nc.tensor.matmul(
                    pv, _mm(et[:, kc, kb * P:(kb + 1) * P]),
                    _mm(wv[:, kc, :]),
                    start=(kc == 0), stop=(kc == KC - 1))
            nc.vector.tensor_copy(v[:, kb, :], pv)

        # x^T becomes dead as a matmul operand now; fold the residual half
        # in place: xt <- 0.5*xt (b0 joins via b0h at the W0 stage).
        for dc in range(KC):
            nc.vector.tensor_scalar_mul(xt[:, dc, :], xt[:, dc, :], 0.5)

        # Absorb outstanding DVE-side psum-slot releases into PE's vector
        # clock: fused-LDW matmuls can carry only ONE sync wait, so any
        # slot whose last accessor was DVE must be re-observed via these tiny
        # matmuls before the attention loop's matmuls touch those slots.
        scr_f = mem.tile([DH + 1, DH], F32, tag="scrf")
        nc.vector.memset(scr_f, 1.0)
        scr = mem.tile([1, 8], BF16, tag="scr")
        nc.vector.tensor_scalar_mul(scr, scr_f[0:1, 0:8], 1.0)
        ones_t = mem.tile([DH + 1, DH], F32R, tag="ones")
        nc.vector.tensor_scalar_mul(ones_t, scr_f, 1.0)
        for _i in range(2):
            dmy = ps2.tile([1, 8], F32, tag="spsum", name="dmy")
            nc.tensor.matmul(dmy, _mm(scr[0:1, 0:1]), _mm(scr), start=True, stop=True)
        dmy2 = psb.tile([1, 8], F32, tag="opsum", name="dmy2")
        nc.tensor.matmul(dmy2, _mm(scr[0:1, 0:1]), _mm(scr), start=True, stop=True)

        # ---- attention per head ------------------------------------------
        # S^T[k,q] = K_h @ Q_h^T; softmax over q = free axis per partition;
        # no max-subtraction (|s| <~ 10 so exp is fp32-safe). D1[k] = rowsum
        # comes free via accum_out. 1/D1 folds into V; a 65th lhsT column of
        # 1/D1 makes psum row 64 the per-q renorm denominator.
        ot = mem.tile([DH, HG, NQ], BF16, tag="ot")
        for h in range(HG):
            hp, off = h // 2, (h % 2) * DH
            po = psb.tile([DH + 1, NK], F32, tag="opsum", name="po")
            for kb in range(NKB):
                e = work.tile([P, NK], BF16, tag="e")
                d1a = small.tile([P, 2], F32, tag="d1a")
                for ck in range(2):
                    ps = ps2.tile([P, NK // 2], F32, tag="spsum", name="ps")
                    for nch in range(2):
                        nc.tensor.matmul(
                            ps[:, nch * 512:(nch + 1) * 512],
                            _mm(kt[off:off + DH, hp, kb * P:(kb + 1) * P]),
                            _mm(qt[off:off + DH, hp,
                                   ck * 1024 + nch * 512:ck * 1024 + (nch + 1) * 512]),
                            start=True, stop=True)
                    nc.scalar.activation(e[:, ck * 1024:(ck + 1) * 1024], ps,
                                         mybir.ActivationFunctionType.Exp,
                                         accum_out=d1a[:, ck:ck + 1])
                rd = small.tile([P, 1], F32, tag="rd")
                nc.vector.tensor_tensor(rd, d1a[:, 0:1], d1a[:, 1:2],
                                        mybir.AluOpType.add)
                nc.vector.reciprocal(rd, rd)
                vaug = small.tile([P, DH + 1], BF16, tag="vaug")
                nc.scalar.activation(vaug[:, :DH], v[:, kb, h * DH:(h + 1) * DH],
                                     mybir.ActivationFunctionType.Copy, scale=rd)
                nc.scalar.copy(vaug[:, DH:DH + 1], rd)
                for nch in range(NCH):
                    nc.tensor.matmul(
                        po[:, nch * 512:(nch + 1) * 512],
                        _mm(vaug), _mm(e[:, nch * 512:(nch + 1) * 512]),
                        start=(kb == 0), stop=(kb == NKB - 1))
            # Drain po on ACT so the psum slot's release is visible through
            # the same ACT wait the next head's PV matmul already needs.
            poc = single.tile([DH + 1, NK], F32R, tag="poc")
            nc.scalar.copy(poc, po)
            # renormalize: O~ = O_raw / denom2. Reciprocal on the denom row,
            # broadcast across 64 partitions with a K=1 ones-matmul (operands
            # at partition 64), multiply into fp32, then round to bf16 for
            # the W0 matmul.
            nc.vector.reciprocal(poc[DH:DH + 1, :], poc[DH:DH + 1, :])
            for ck in range(NCH):
                rb = ps2.tile([DH, 512], F32, tag="spsum", name="rb")
                nc.tensor.matmul(rb, _mm(ones_t[DH:DH + 1, :]),
                                 _mm(poc[DH:DH + 1, ck * 512:(ck + 1) * 512]),
                                 start=True, stop=True)
                otf = work.tile([DH, 512], F32, tag="fout", name="otf")
                nc.vector.tensor_tensor(otf, poc[:DH, ck * 512:(ck + 1) * 512],
                                        rb, mybir.AluOpType.mult)
                nc.vector.tensor_scalar_mul(ot[:, h, ck * 512:(ck + 1) * 512],
                                            otf, 1.0)

        # absorb attention-era slot releases before the W0 matmuls
        for _i in range(2):
            dmy3 = ps2.tile([1, 8], F32, tag="spsum", name="dmy3")
            nc.tensor.matmul(dmy3, _mm(scr[0:1, 0:1]), _mm(scr), start=True, stop=True)

        # ---- W0 partial + residual half: per core 0.5*x - 0.5*b0 - A_part
        # so the pair ReduceScatter(add) directly yields R = x - (O@W0+b0).
        at = mem.tile([P, KC, NQ], BF16, tag="bigB")  # reuses et's slot
        a_part = dram.tile([2, D, QH], BF16, tag="a_part")
        for dc in range(KC):
            for nch in range(NCH):
                pa = ps2.tile([P, 512], F32, tag="spsum", name="pa")
                for h in range(HG):
                    nc.tensor.matmul(
                        pa, _mm(w0[:, h, dc * P:(dc + 1) * P]),
                        _mm(ot[:, h, nch * 512:(nch + 1) * 512]),
                        start=(h == 0), stop=(h == HG - 1))
                ts = work.tile([P, 512], F32, tag="ts", name="ts")
                nc.vector.tensor_scalar(ts, pa, scalar1=b0h[:, dc:dc + 1],
                                        scalar2=None, op0=mybir.AluOpType.add)
                nc.vector.tensor_tensor(at[:, dc, nch * 512:(nch + 1) * 512],
                                        xt[:, dc, nch * 512:(nch + 1) * 512],
                                        ts, mybir.AluOpType.subtract)
        for s in range(2):  # one DMA per RS slot keeps the collective's waits low
            nc.sync.dma_start(
                a_part[s].rearrange("(c p) q -> p c q", p=P),
                at[:, :, s * QH:(s + 1) * QH])

        a_rs = dram.tile([D, QH], BF16, tag="a_rs")
        nc.gpsimd.collective_compute(
            "ReduceScatter", mybir.AluOpType.add,
            replica_groups=[[0, 1], [2, 3], [4, 5], [6, 7]],
            ins=[a_part.opt()], outs=[a_rs.opt()])

        # ---- W1 on local q-half ------------------------------------------
        # RS rank r gets slot r = q rows [r*QH,(r+1)*QH); rank == head-group
        # g, so core 2b+g owns output rows [g*QH,(g+1)*QH) of batch b.
        rt = mem.tile([P, KC, QH], BF16, tag="kt")  # reuses kt's slot
        nc.sync.dma_start(rt, a_rs[:].rearrange("(c p) q -> p c q", p=P))
        for dc in range(KC):
            nc.vector.tensor_scalar_mul(rt[:, dc, :], rt[:, dc, :], 1.0)
        for mq in range(QH // P):
            pf = ps2.tile([P, D], F32, tag="spsum", name="pf")
            for kc in range(KC):
                nc.tensor.matmul(pf, _mm(rt[:, kc, mq * P:(mq + 1) * P]),
                                 _mm(w1[:, kc, :]),
                                 start=(kc == 0), stop=(kc == KC - 1))
            fo = work.tile([P, D], BF16, tag="fo", name="fo")
            nc.vector.tensor_tensor(fo, pf, b1b, mybir.AluOpType.add)
            nc.sync.dma_start(out_d[mq * P:(mq + 1) * P, :], fo)

    _strip_redundant_self_waits(nc)
    _keep_latest_wait_only(nc)
    return nc


def _keep_latest_wait_only(nc):
    """Under linearize=True every instruction syncs on its predecessor, so
    waits on earlier instructions are transitively covered; keep only the
    wait whose target is latest in program order (walrus on this toolchain
    encodes a single sync wait per engine instruction)."""
    insts = []
    for blk in nc.m.functions[0].blocks:
        insts.extend(blk.instructions)
    pos = {}
    cums = {}
    for i, inst in enumerate(insts):
        si = getattr(inst, 'sync_info', None)
        if si and si.on_update:
            for u in si.on_update:
                cums[u.ant_name] = cums.get(u.ant_name, 0) + u.update_value
                pos[(u.ant_name, cums[u.ant_name])] = i
    for inst in insts:
        si = getattr(inst, 'sync_info', None)
        if si is None or not si.on_wait or len(si.on_wait) < 2:
            continue
        ws = list(si.on_wait)
        ws.sort(key=lambda w: pos.get((w.ant_name, w.wait_value), -1))
        si.on_wait = [ws[-1]]


_ENGINE_SEMS = {"PE_44", "Activation_44", "DVE_44", "Pool_44", "SP_44"}


def _strip_redundant_self_waits(nc):
    """Drop same-engine self waits: these engines retire instructions in
    pc order (strict FIFO queues; PE matmul completions are pc-monotone),
    so an instruction never needs a semaphore wait on its own engine's
    earlier non-DMA instruction. Needed because walrus encodes very few
    sync waits per instruction (1 for fused-LDW matmuls and ACTIVATE)."""
    insts = []
    for blk in nc.m.functions[0].blocks:
        insts.extend(blk.instructions)
    ticks = {s: {} for s in _ENGINE_SEMS}
    cums = {s: 0 for s in _ENGINE_SEMS}
    for inst in insts:
        si = getattr(inst, 'sync_info', None)
        if si and si.on_update:
            for u in si.on_update:
                if u.ant_name in _ENGINE_SEMS:
                    cums[u.ant_name] += u.update_value
                    ticks[u.ant_name][cums[u.ant_name]] = inst
    for inst in insts:
        tname = type(inst).__name__
        if 'DMA' in tname or 'Collective' in tname:
            continue
        si = getattr(inst, 'sync_info', None)
        if si is None or not si.on_wait or len(si.on_wait) < 2:
            continue
        my_engine = getattr(inst, 'engine', None)
        kept = []
        for w in si.on_wait:
            tgt = ticks.get(w.ant_name, {}).get(w.wait_value)
            same_engine = (
                tgt is not None
                and 'DMA' not in type(tgt).__name__
                and 'Collective' not in type(tgt).__name__
                and getattr(tgt, 'engine', None) == my_engine
            )
            if not same_engine:
                kept.append(w)
        if len(kept) != len(si.on_wait):
            si.on_wait = kept


def make_in_maps(init_query, embedding, Wq, Wk, Wv, W0, b0, W1, b1):
    init_query = np.asarray(init_query, np.float32)
    embedding = np.asarray(embedding, np.float32)
    Wq, Wk, Wv = (np.asarray(a, np.float32) for a in (Wq, Wk, Wv))
    W0, W1 = np.asarray(W0, np.float32), np.asarray(W1, np.float32)
    b0 = np.asarray(b0, np.float32)
    b1 = np.asarray(b1, np.float32)

    xT = [np.ascontiguousarray(init_query[b].T).astype(NPBF) for b in range(B)]
    eT = [np.ascontiguousarray(embedding[b].T).astype(NPBF) for b in range(B)]
    whalves = []  # whalves[g][quarter] -> flat bf16 SEGB
    for g in range(2):
        cols = slice(g * GCOL, (g + 1) * GCOL)
        whalves.append([
            np.ascontiguousarray(Wq[:, cols]).astype(NPBF).ravel(),
            np.ascontiguousarray(Wk[:, cols]).astype(NPBF).ravel(),
            np.ascontiguousarray(Wv[:, cols]).astype(NPBF).ravel(),
            np.ascontiguousarray(W0[cols, :]).astype(NPBF).ravel(),
        ])
    w8 = np.concatenate([W1.astype(NPBF).ravel(),
                         b0.astype(NPBF), b1.astype(NPBF)])
    in_maps = []
    for c in range(8):
        b, g = c // 2, c % 2
        qs = slice(g * QH, (g + 1) * QH)
        blob = np.concatenate([
            xT[b][:, qs].ravel(), eT[b][:, qs].ravel(),
            whalves[g][b],
            w8[c * SEGC:(c + 1) * SEGC],
        ])
        assert blob.shape == (NIN,)
        in_maps.append({"inp": blob})
    return in_maps


def kernel(init_query, embedding, Wq, Wk, Wv, W0, b0, W1, b1):
    nc = build_kernel()
    in_maps = make_in_maps(init_query, embedding, Wq, Wk, Wv, W0, b0, W1, b1)
    res = run_bass_kernel_spmd(nc, in_maps, list(range(8)))
    out = np.empty((B, NQ, D), np.float32)
    for c in range(8):
        b, g = c // 2, c % 2
        out[b, g * QH:(g + 1) * QH, :] = np.asarray(
            res.results[c]["out"]).astype(np.float32)
    return out


# revision 3
# speedup vs baseline: 1.5620x; 1.3096x over previous
"""CrossAttention (softmax over query axis + row renorm) on 8 trn2 cores.

Wire-optimized vs v1: the axon tunnel (~30 MB/s) dominates wall time, so
inputs are shipped exactly once in bf16 (2.43 MB/core = unique/8) and
reconstructed on device with three AllGathers over NeuronLink:
  segA (pair AG   [2b,2b+1]): this core's q-half of x^T[b] and e^T[b]
  segB (AG-4 [g,2+g,4+g,6+g]): one of {Wq,Wk,Wv,W0} column-halves for g
  segC (AG-8): 1/8 of W1 + b0 + b1
Sharding: core c -> batch b = c//2, head-group g = c%2 (4 of 8 heads).
Compute: Q/K/V projections (bf16 matmuls), attention with q-axis softmax
in S^T=[k,q] layout, W0 partial folded with the residual (each core
contributes 0.5*x - 0.5*b0 - A_part so the 2-core bf16 ReduceScatter
yields R = x - (O@W0 + b0) directly), then W1 on the local q-half.
Output: bf16 [QH, D] = rows [g*1024:(g+1)*1024] of batch b.

Shapes (hardcoded): B=4, NQ=NK=2048, D=512, H=8, DH=64.
"""

import sys

for p in ("/opt/trn_rl_repo", "/opt/pypackages"):
    if p not in sys.path:
        sys.path.insert(0, p)

import numpy as np
import ml_dtypes
from contextlib import ExitStack

import concourse.bass as bass
import concourse.mybir as mybir
import concourse.tile as tile
from concourse.bass_utils import run_bass_kernel_spmd

try:  # persistent XLA cache: the per-call jit closure otherwise recompiles
    import jax
    jax.config.update("jax_compilation_cache_dir", "/tmp/jaxcache")
    jax.config.update("jax_persistent_cache_min_compile_time_secs", 0.0)
except Exception:
    pass

B, NQ, NK, D, H, DH = 4, 2048, 2048, 512, 8, 64
HG = 4          # heads per core (head-group size)
GCOL = HG * DH  # 256 projection columns per core
QH = NQ // 2    # query rows per core after reduce-scatter
P = 128
F32 = mybir.dt.float32
F32R = mybir.dt.float32r
BF16 = mybir.dt.bfloat16
U8 = mybir.dt.uint8
F8 = mybir.dt.float8e3       # e3m4: 4 mantissa bits
NPBF = ml_dtypes.bfloat16
NPF8 = ml_dtypes.float8_e3m4
WSCALE = 64.0                # host-prescale for fp8 wk/wv (dodges subnormals)

# byte sizes of the three wire segments (per core)
XB = D * QH * 2              # 1,048,576 B: x^T half, bf16
EB = D * QH                  #   524,288 B: e^T half, fp8 e3m4
SEGA = XB + EB               # 1,572,864 B
WQB = D * GCOL * 2           #   262,144 B: wq half, bf16
W0B = GCOL * D * 2           #   262,144 B: w0 half, bf16
WKB = D * GCOL               #   131,072 B: wk half, fp8
WVB = D * GCOL               #   131,072 B: wv half, fp8
SEGB = (WQB + W0B + WKB + WVB) // 4   # 196,608 B: quarter of the stream
W1B = D * D * 2              #   524,288 B: w1, bf16
SEGC = (W1B + 4 * D) // 8    #    65,792 B: 1/8 of (w1, b0, b1) bf16
NIN = SEGA + SEGB + SEGC     # 1,835,264 B per core

LINEARIZE = True  # serialize scheduling: walrus encodes only 1 sync wait per
                  # engine instruction on this toolchain; the overlap-scheduled
                  # build trips 'Too many sync wait commands' in codegen


def _mm(ap):
    return ap


def build_kernel():
    nc = bass.Bass(num_devices=8)

    in_d = nc.dram_tensor("inp", [NIN], U8, kind="ExternalInput")
    out_d = nc.dram_tensor("out", [QH, D], BF16, kind="ExternalOutput")

    KC = D // P      # 4 contraction subtiles of 128
    NKB = NK // P    # 16 key blocks
    NCH = NK // 512  # 4 free-dim chunks of 512 over q/k

    with tile.TileContext(nc, linearize=LINEARIZE) as tc, ExitStack() as ctx, \
            nc.allow_low_precision(reason="bf16 wire/matmul; tolerance 2e-2"):
        mem = ctx.enter_context(tc.tile_pool(name="mem", bufs=1))
        work = ctx.enter_context(tc.tile_pool(name="work", bufs=2))
        single = ctx.enter_context(tc.tile_pool(name="single", bufs=1))
        small = ctx.enter_context(tc.tile_pool(name="small", bufs=4))
        ps2 = ctx.enter_context(tc.tile_pool(name="ps2", bufs=2, space="PSUM"))
        psb = ctx.enter_context(tc.tile_pool(name="psb", bufs=1, space="PSUM"))
        dram = ctx.enter_context(tc.tile_pool(name="dram", bufs=1, space="DRAM"))

        # ---- distribute the packed input with AllGathers -----------------
        stA = mem.tile([P, SEGA // P], U8, tag="stA")
        stB = mem.tile([P, SEGB // P], U8, tag="stB")
        stC = mem.tile([P, SEGC // P], U8, tag="stC")
        nc.sync.dma_start(stA, in_d[0:SEGA].rearrange("(p f) -> p f", p=P))
        nc.sync.dma_start(stB, in_d[SEGA:SEGA + SEGB].rearrange("(p f) -> p f", p=P))
        nc.sync.dma_start(stC, in_d[SEGA + SEGB:NIN].rearrange("(p f) -> p f", p=P))
        aginA = dram.tile([SEGA], U8, tag="aginA")
        aginB = dram.tile([SEGB], U8, tag="aginB")
        aginC = dram.tile([SEGC], U8, tag="aginC")
        nc.sync.dma_start(aginA.rearrange("(p f) -> p f", p=P), stA)
        nc.sync.dma_start(aginB.rearrange("(p f) -> p f", p=P), stB)
        nc.sync.dma_start(aginC.rearrange("(p f) -> p f", p=P), stC)
        agA = dram.tile([2 * SEGA], U8, tag="agA")
        agB = dram.tile([4 * SEGB], U8, tag="agB")
        agC = dram.tile([8 * SEGC], U8, tag="agC")
        nc.gpsimd.collective_compute(
            "AllGather", mybir.AluOpType.bypass,
            replica_groups=[[0, 1], [2, 3], [4, 5], [6, 7]],
            ins=[aginA.opt()], outs=[agA.opt()])
        nc.gpsimd.collective_compute(
            "AllGather", mybir.AluOpType.bypass,
            replica_groups=[[0, 2, 4, 6], [1, 3, 5, 7]],
            ins=[aginB.opt()], outs=[agB.opt()])
        nc.gpsimd.collective_compute(
            "AllGather", mybir.AluOpType.bypass,
            replica_groups=[[0, 1, 2, 3, 4, 5, 6, 7]],
            ins=[aginC.opt()], outs=[agC.opt()])

        # ---- unpack gathered tensors into SBUF ---------------------------
        # agA slot s (pair rank s = head-group s) holds q columns
        # [s*1024,(s+1)*1024) of x^T[b] (bf16) then e^T[b] (fp8),
        # each laid out (c p q) with c=4, p=128.
        xt = mem.tile([P, KC, NQ], BF16, tag="bigA")
        et = mem.tile([P, KC, NK], F8, tag="bigB")
        for s in range(2):
            o = s * SEGA
            nc.sync.dma_start(
                xt[:, :, s * QH:(s + 1) * QH],
                agA[o:o + XB].bitcast(BF16).rearrange("(c p q) -> p c q", p=P, q=QH))
            nc.sync.dma_start(
                et[:, :, s * QH:(s + 1) * QH],
                agA[o + XB:o + XB + EB].bitcast(F8).rearrange("(c p q) -> p c q", p=P, q=QH))
        # agB stream bytes: wq_h bf16 | w0_h bf16 | wk_h fp8 | wv_h fp8
        wq = mem.tile([P, KC, GCOL], BF16, tag="wq")
        wk = mem.tile([P, KC, GCOL], F8, tag="wk")
        wv = mem.tile([P, KC, GCOL], F8, tag="wv")
        nc.sync.dma_start(wq, agB[0:WQB].bitcast(BF16).rearrange("(c p m) -> p c m", p=P, m=GCOL))
        w0 = mem.tile([DH, HG, D], BF16, tag="w0")
        nc.sync.dma_start(w0, agB[WQB:WQB + W0B].bitcast(BF16).rearrange("(h p d) -> p h d", p=DH, d=D))
        nc.sync.dma_start(wk, agB[WQB + W0B:WQB + W0B + WKB].bitcast(F8).rearrange("(c p m) -> p c m", p=P, m=GCOL))
        nc.sync.dma_start(wv, agB[WQB + W0B + WKB:WQB + W0B + WKB + WVB].bitcast(F8).rearrange("(c p m) -> p c m", p=P, m=GCOL))
        # agC stream bytes: w1 (c p d) bf16 | b0 (c p) bf16 | b1 (d) bf16
        w1 = mem.tile([P, KC, D], BF16, tag="w1")
        nc.sync.dma_start(w1, agC[0:W1B].bitcast(BF16).rearrange("(c p d) -> p c d", p=P, d=D))
        b0s = mem.tile([P, KC], BF16, tag="b0")
        nc.sync.dma_start(b0s, agC[W1B:W1B + 2 * D].bitcast(BF16).rearrange("(c p) -> p c", p=P))
        b1b = mem.tile([P, D], BF16, tag="b1")
        nc.gpsimd.dma_start(b1b, agC[W1B + 2 * D:W1B + 4 * D].bitcast(BF16).partition_broadcast(P))

        # DVE in-place x1.0 "rounding" passes: make DVE the single producer
        # proc of every matmul operand (fused-LDW matmuls carry only one
        # sync wait, so each matmul may depend on at most one engine).
        for t in (xt, et, wq, wk, wv, w0, w1):
            nc.vector.tensor_scalar_mul(t, t, 1.0)
        b0h = mem.tile([P, KC], F32, tag="b0h")  # 0.5*b0 per partition
        nc.vector.tensor_scalar_mul(b0h, b0s, 0.5)

        # ---- projections: QT/KT [128(head pair), 2, N*], V [128, 16, GCOL]
        qt = mem.tile([P, 2, NQ], BF16, tag="qt")
        kt = mem.tile([P, 2, NK], BF16, tag="kt")
        for mc in range(2):        # two head-pairs: 128 cols of wq each
            for nch in range(NCH):
                pq = ps2.tile([P, 512], F32, tag="spsum", name="pq")
                pk = ps2.tile([P, 512], F32, tag="spsum", name="pk")
                for kc in range(KC):
                    nc.tensor.matmul(
                        pq, _mm(wq[:, kc, mc * P:(mc + 1) * P]),
                        _mm(xt[:, kc, nch * 512:(nch + 1) * 512]),
                        start=(kc == 0), stop=(kc == KC - 1))
                for kc in range(KC):
                    nc.tensor.matmul(
                        pk, _mm(wk[:, kc, mc * P:(mc + 1) * P]),
                        _mm(et[:, kc, nch * 512:(nch + 1) * 512]),
                        start=(kc == 0), stop=(kc == KC - 1))
                nc.vector.tensor_copy(qt[:, mc, nch * 512:(nch + 1) * 512], pq)
                nc.vector.tensor_scalar_mul(kt[:, mc, nch * 512:(nch + 1) * 512],
                                            pk, 1.0 / WSCALE)

        v = mem.tile([P, NKB, GCOL], BF16, tag="v")
        for kb in range(NKB):
            pv = ps2.tile([P, GCOL], F32, tag="spsum", name="pv")
            for kc in range(KC):
                nc.tensor.matmul(
                    pv, _mm(et[:, kc, kb * P:(kb + 1) * P]),
                    _mm(wv[:, kc, :]),
                    start=(kc == 0), stop=(kc == KC - 1))
            nc.vector.tensor_scalar_mul(v[:, kb, :], pv, 1.0 / WSCALE)

        # x^T becomes dead as a matmul operand now; fold the residual half
        # in place: xt <- 0.5*xt (b0 joins via b0h at the W0 stage).
        for dc in range(KC):
            nc.vector.tensor_scalar_mul(xt[:, dc, :], xt[:, dc, :], 0.5)

        # Absorb outstanding DVE-side psum-slot releases into PE's vector
        # clock: fused-LDW matmuls can carry only ONE sync wait, so any
        # slot whose last accessor was DVE must be re-observed via these tiny
        # matmuls before the attention loop's matmuls touch those slots.
        scr_f = mem.tile([DH + 1, DH], F32, tag="scrf")
        nc.vector.memset(scr_f, 1.0)
        scr = mem.tile([1, 8], BF16, tag="scr")
        nc.vector.tensor_scalar_mul(scr, scr_f[0:1, 0:8], 1.0)
        ones_t = mem.tile([DH + 1, DH], F32R, tag="ones")
        nc.vector.tensor_scalar_mul(ones_t, scr_f, 1.0)
        for _i in range(2):
            dmy = ps2.tile([1, 8], F32, tag="spsum", name="dmy")
            nc.tensor.matmul(dmy, _mm(scr[0:1, 0:1]), _mm(scr), start=True, stop=True)
        dmy2 = psb.tile([1, 8], F32, tag="opsum", name="dmy2")
        nc.tensor.matmul(dmy2, _mm(scr[0:1, 0:1]), _mm(scr), start=True, stop=True)

        # ---- attention per head ------------------------------------------
        # S^T[k,q] = K_h @ Q_h^T; softmax over q = free axis per partition;
        # no max-subtraction (|s| <~ 10 so exp is fp32-safe). D1[k] = rowsum
        # comes free via accum_out. 1/D1 folds into V; a 65th lhsT column of
        # 1/D1 makes psum row 64 the per-q renorm denominator.
        ot = mem.tile([DH, HG, NQ], BF16, tag="ot")
        for h in range(HG):
            hp, off = h // 2, (h % 2) * DH
            po = psb.tile([DH + 1, NK], F32, tag="opsum", name="po")
            for kb in range(NKB):
                e = work.tile([P, NK], BF16, tag="e")
                d1a = small.tile([P, 2], F32, tag="d1a")
                for ck in range(2):
                    ps = ps2.tile([P, NK // 2], F32, tag="spsum", name="ps")
                    for nch in range(2):
                        nc.tensor.matmul(
                            ps[:, nch * 512:(nch + 1) * 512],
                            _mm(kt[off:off + DH, hp, kb * P:(kb + 1) * P]),
                            _mm(qt[off:off + DH, hp,
                                   ck * 1024 + nch * 512:ck * 1024 + (nch + 1) * 512]),
                            start=True, stop=True)
                    nc.scalar.activation(e[:, ck * 1024:(ck + 1) * 1024], ps,
                                         mybir.ActivationFunctionType.Exp,
                                         accum_out=d1a[:, ck:ck + 1])
                rd = small.tile([P, 1], F32, tag="rd")
                nc.vector.tensor_tensor(rd, d1a[:, 0:1], d1a[:, 1:2],
                                        mybir.AluOpType.add)
                nc.vector.reciprocal(rd, rd)
                vaug = small.tile([P, DH + 1], BF16, tag="vaug")
                nc.scalar.activation(vaug[:, :DH], v[:, kb, h * DH:(h + 1) * DH],
                                     mybir.ActivationFunctionType.Copy, scale=rd)
                nc.scalar.copy(vaug[:, DH:DH + 1], rd)
                for nch in range(NCH):
                    nc.tensor.matmul(
                        po[:, nch * 512:(nch + 1) * 512],
                        _mm(vaug), _mm(e[:, nch * 512:(nch + 1) * 512]),
                        start=(kb == 0), stop=(kb == NKB - 1))
            # Drain po on ACT so the psum slot's release is visible through
            # the same ACT wait the next head's PV matmul already needs.
            poc = single.tile([DH + 1, NK], F32R, tag="poc")
            nc.scalar.copy(poc, po)
            # renormalize: O~ = O_raw / denom2. Reciprocal on the denom row,
            # broadcast across 64 partitions with a K=1 ones-matmul (operands
            # at partition 64), multiply into fp32, then round to bf16 for
            # the W0 matmul.
            nc.vector.reciprocal(poc[DH:DH + 1, :], poc[DH:DH + 1, :])
            for ck in range(NCH):
                rb = ps2.tile([DH, 512], F32, tag="spsum", name="rb")
                nc.tensor.matmul(rb, _mm(ones_t[DH:DH + 1, :]),
                                 _mm(poc[DH:DH + 1, ck * 512:(ck + 1) * 512]),
                                 start=True, stop=True)
                otf = work.tile([DH, 512], F32, tag="fout", name="otf")
                nc.vector.tensor_tensor(otf, poc[:DH, ck * 512:(ck + 1) * 512],
                                        rb, mybir.AluOpType.mult)
                nc.vector.tensor_scalar_mul(ot[:, h, ck * 512:(ck + 1) * 512],
                                            otf, 1.0)

        # absorb attention-era slot releases before the W0 matmuls
        for _i in range(2):
            dmy3 = ps2.tile([1, 8], F32, tag="spsum", name="dmy3")
            nc.tensor.matmul(dmy3, _mm(scr[0:1, 0:1]), _mm(scr), start=True, stop=True)

        # ---- W0 partial + residual half: per core 0.5*x - 0.5*b0 - A_part
        # so the pair ReduceScatter(add) directly yields R = x - (O@W0+b0).
        at = mem.tile([P, KC, NQ], BF16, tag="bigB")  # reuses et's slot
        a_part = dram.tile([2, D, QH], BF16, tag="a_part")
        for dc in range(KC):
            for nch in range(NCH):
                pa = ps2.tile([P, 512], F32, tag="spsum", name="pa")
                for h in range(HG):
                    nc.tensor.matmul(
                        pa, _mm(w0[:, h, dc * P:(dc + 1) * P]),
                        _mm(ot[:, h, nch * 512:(nch + 1) * 512]),
                        start=(h == 0), stop=(h == HG - 1))
                ts = work.tile([P, 512], F32, tag="ts", name="ts")
                nc.vector.tensor_scalar(ts, pa, scalar1=b0h[:, dc:dc + 1],
                                        scalar2=None, op0=mybir.AluOpType.add)
                nc.vector.tensor_tensor(at[:, dc, nch * 512:(nch + 1) * 512],
                                        xt[:, dc, nch * 512:(nch + 1) * 512],
                                        ts, mybir.AluOpType.subtract)
        for s in range(2):  # one DMA per RS slot keeps the collective's waits low
            nc.sync.dma_start(
                a_part[s].rearrange("(c p) q -> p c q", p=P),
                at[:, :, s * QH:(s + 1) * QH])

        a_rs = dram.tile([D, QH], BF16, tag="a_rs")
        nc.gpsimd.collective_compute(
            "ReduceScatter", mybir.AluOpType.add,
            replica_groups=[[0, 1], [2, 3], [4, 5], [6, 7]],
            ins=[a_part.opt()], outs=[a_rs.opt()])

        # ---- W1 on local q-half ------------------------------------------
        # RS rank r gets slot r = q rows [r*QH,(r+1)*QH); rank == head-group
        # g, so core 2b+g owns output rows [g*QH,(g+1)*QH) of batch b.
        rt = mem.tile([P, KC, QH], BF16, tag="kt")  # reuses kt's slot
        nc.sync.dma_start(rt, a_rs[:].rearrange("(c p) q -> p c q", p=P))
        for dc in range(KC):
            nc.vector.tensor_scalar_mul(rt[:, dc, :], rt[:, dc, :], 1.0)
        for mq in range(QH // P):
            pf = ps2.tile([P, D], F32, tag="spsum", name="pf")
            for kc in range(KC):
                nc.tensor.matmul(pf, _mm(rt[:, kc, mq * P:(mq + 1) * P]),
                                 _mm(w1[:, kc, :]),
                                 start=(kc == 0), stop=(kc == KC - 1))
            fo = work.tile([P, D], BF16, tag="fo", name="fo")
            nc.vector.tensor_tensor(fo, pf, b1b, mybir.AluOpType.add)
            nc.sync.dma_start(out_d[mq * P:(mq + 1) * P, :], fo)

    _strip_redundant_self_waits(nc)
    _keep_latest_wait_only(nc)
    return nc


def _keep_latest_wait_only(nc):
    """Under linearize=True every instruction syncs on its predecessor, so
    waits on earlier instructions are transitively covered; keep only the
    wait whose target is latest in program order (walrus on this toolchain
    encodes a single sync wait per engine instruction)."""
    insts = []
    for blk in nc.m.functions[0].blocks:
        insts.extend(blk.instructions)
    pos = {}
    cums = {}
    for i, inst in enumerate(insts):
        si = getattr(inst, 'sync_info', None)
        if si and si.on_update:
            for u in si.on_update:
                cums[u.ant_name] = cums.get(u.ant_name, 0) + u.update_value
                pos[(u.ant_name, cums[u.ant_name])] = i
    for inst in insts:
        si = getattr(inst, 'sync_info', None)
        if si is None or not si.on_wait or len(si.on_wait) < 2:
            continue
        ws = list(si.on_wait)
        ws.sort(key=lambda w: pos.get((w.ant_name, w.wait_value), -1))
        si.on_wait = [ws[-1]]


_ENGINE_SEMS = {"PE_44", "Activation_44", "DVE_44", "Pool_44", "SP_44"}


def _strip_redundant_self_waits(nc):
    """Drop same-engine self waits: these engines retire instructions in
    pc order (strict FIFO queues; PE matmul completions are pc-monotone),
    so an instruction never needs a semaphore wait on its own engine's
    earlier non-DMA instruction. Needed because walrus encodes very few
    sync waits per instruction (1 for fused-LDW matmuls and ACTIVATE)."""
    insts = []
    for blk in nc.m.functions[0].blocks:
        insts.extend(blk.instructions)
    ticks = {s: {} for s in _ENGINE_SEMS}
    cums = {s: 0 for s in _ENGINE_SEMS}
    for inst in insts:
        si = getattr(inst, 'sync_info', None)
        if si and si.on_update:
            for u in si.on_update:
                if u.ant_name in _ENGINE_SEMS:
                    cums[u.ant_name] += u.update_value
                    ticks[u.ant_name][cums[u.ant_name]] = inst
    for inst in insts:
        tname = type(inst).__name__
        if 'DMA' in tname or 'Collective' in tname:
            continue
        si = getattr(inst, 'sync_info', None)
        if si is None or not si.on_wait or len(si.on_wait) < 2:
            continue
        my_engine = getattr(inst, 'engine', None)
        kept = []
        for w in si.on_wait:
            tgt = ticks.get(w.ant_name, {}).get(w.wait_value)
            same_engine = (
                tgt is not None
                and 'DMA' not in type(tgt).__name__
                and 'Collective' not in type(tgt).__name__
                and getattr(tgt, 'engine', None) == my_engine
            )
            if not same_engine:
                kept.append(w)
        if len(kept) != len(si.on_wait):
            si.on_wait = kept


def _u8(a):
    return np.ascontiguousarray(a).view(np.uint8).ravel()


def make_in_maps(init_query, embedding, Wq, Wk, Wv, W0, b0, W1, b1):
    init_query = np.asarray(init_query, np.float32)
    embedding = np.asarray(embedding, np.float32)
    Wq, Wk, Wv = (np.asarray(a, np.float32) for a in (Wq, Wk, Wv))
    W0, W1 = np.asarray(W0, np.float32), np.asarray(W1, np.float32)
    b0 = np.asarray(b0, np.float32)
    b1 = np.asarray(b1, np.float32)

    xT = [np.ascontiguousarray(init_query[b].T).astype(NPBF) for b in range(B)]
    eT = [np.ascontiguousarray(embedding[b].T).astype(NPF8) for b in range(B)]
    wbytes = []  # wbytes[g]: byte stream wq_h bf16 | w0_h bf16 | wk_h fp8 | wv_h fp8
    for g in range(2):
        cols = slice(g * GCOL, (g + 1) * GCOL)
        wbytes.append(np.concatenate([
            _u8(np.ascontiguousarray(Wq[:, cols]).astype(NPBF)),
            _u8(np.ascontiguousarray(W0[cols, :]).astype(NPBF)),
            _u8((np.ascontiguousarray(Wk[:, cols]) * WSCALE).astype(NPF8)),
            _u8((np.ascontiguousarray(Wv[:, cols]) * WSCALE).astype(NPF8)),
        ]))
    w8 = np.concatenate([_u8(W1.astype(NPBF)), _u8(b0.astype(NPBF)),
                         _u8(b1.astype(NPBF))])
    in_maps = []
    for c in range(8):
        b, g = c // 2, c % 2
        qs = slice(g * QH, (g + 1) * QH)
        blob = np.concatenate([
            _u8(xT[b][:, qs]), _u8(eT[b][:, qs]),
            wbytes[g][b * SEGB:(b + 1) * SEGB],
            w8[c * SEGC:(c + 1) * SEGC],
        ])
        assert blob.shape == (NIN,)
        in_maps.append({"inp": blob})
    return in_maps


def kernel(init_query, embedding, Wq, Wk, Wv, W0, b0, W1, b1):
    nc = build_kernel()
    in_maps = make_in_maps(init_query, embedding, Wq, Wk, Wv, W0, b0, W1, b1)
    res = run_bass_kernel_spmd(nc, in_maps, list(range(8)))
    out = np.empty((B, NQ, D), np.float32)
    for c in range(8):
        b, g = c // 2, c % 2
        out[b, g * QH:(g + 1) * QH, :] = np.asarray(
            res.results[c]["out"]).astype(np.float32)
    return out
